# revision 11
# baseline (speedup 1.0000x reference)
"""DisenVAE forward pass on 8 Trainium2 NeuronCores.

Data-parallel over the batch (1024 rows / 8 cores = 128 rows each);
encoder/decoder weights + item/core codebooks replicated per core.

Math (per core, n=128 local rows, M=20000, K=7, D=100, H=300, TAU=0.1):
  items_n = l2norm(items, rows); cores_n = l2norm(cores, rows)
  cates   = softmax_k(items_n @ cores_n.T / TAU)                  [M, K]
  h       = tanh((X*catesT broadcast) @ W1 + b1) @ W2 + b2        [n*K, 2D]
  mu, logvar = h[:, :D], h[:, D:]
  z_n     = l2norm(mu)
  probs   = sum_k exp(z_n @ items_n.T / TAU) * catesT             [n, M]
  logits  = log_softmax(log(probs))

Key algebraic folds used on device:
  * cates scaling of X is applied as per-partition tensor_scalar on X^T
    tiles: Xk^T[m,i] = X[i,m] * (1/s[m]) * ec[m,k] where ec = exp(ct/TAU)
    and s = sum_k ec (softmax denominator), so no [n*K, M] intermediate.
  * decoder: sum_k exp(z_n@items_n.T/TAU)*cates = sum_k exp(invn10[m] *
    items[m,:] @ (z_n + cores_n)[:, (k,i)] + ln(1/s[m])) -- the cores_n term
    reproduces the cates numerator, the bias the denominator, so a single
    [100, 896] moving operand serves the whole decoder and the per-k mixture
    weights cost nothing extra.
  * log_softmax(log p) = ln(p * (1/S)) with S = rowsum(p): one activation.
"""

import numpy as np
import ml_dtypes

import concourse.bass as bass
import concourse.tile as tile
from concourse import bacc
from concourse import mybir
from concourse.bass_utils import run_bass_kernel_spmd

# ---- problem shapes (hardcoded per contest contract) ----
N_CORES = 8
N, M, K, D, H = 1024, 20000, 7, 100, 300
DD = 2 * D
NL = N // N_CORES        # 128 batch rows per core
CS = 125                 # codebook chunk (m) size
NCH = M // CS            # 160 chunks
KW = K * NL              # 896 decoder free width
TAU = 0.1

F32 = mybir.dt.float32
F32R = mybir.dt.float32r
BF16 = mybir.dt.bfloat16

# knobs
PH1_BF16 = False         # phase-1 GEMM in bf16 (faster, lower precision)


def build_nc():
    nc = bacc.Bacc("TRN2", target_bir_lowering=False, debug=False,
                   num_devices=N_CORES)

    w1_dt = BF16 if PH1_BF16 else F32R

    # ---- DRAM I/O ----
    Xl = nc.dram_tensor("Xl", [NL, M], F32, kind="ExternalInput").ap()
    W1d = nc.dram_tensor("W1d", [M, H], w1_dt, kind="ExternalInput").ap()
    b1r = nc.dram_tensor("b1r", [1, H], F32R, kind="ExternalInput").ap()
    W2p = nc.dram_tensor("W2p", [3, 128, 256], F32R, kind="ExternalInput").ap()
    b2r = nc.dram_tensor("b2r", [1, 256], F32R, kind="ExternalInput").ap()
    itemsd = nc.dram_tensor("itemsd", [M, D], F32, kind="ExternalInput").ap()
    itemsTd = nc.dram_tensor("itemsTd", [D, M], F32R, kind="ExternalInput").ap()
    coresd = nc.dram_tensor("coresd", [K, D], F32, kind="ExternalInput").ap()
    idnd = nc.dram_tensor("idnd", [128, 128], F32, kind="ExternalInput").ap()
    idnbd = nc.dram_tensor("idnbd", [128, 128], BF16, kind="ExternalInput").ap()
    onesfd = nc.dram_tensor("onesfd", [1, 128], F32R, kind="ExternalInput").ap()
    onesbd = nc.dram_tensor("onesbd", [128, 1], BF16, kind="ExternalInput").ap()
    indd = nc.dram_tensor("indd", [K, KW], F32R, kind="ExternalInput").ap()

    lg = nc.dram_tensor("lg", [NL, M], F32, kind="ExternalOutput").ap()
    muo = nc.dram_tensor("muo", [NL * K, D], F32, kind="ExternalOutput").ap()
    lvo = nc.dram_tensor("lvo", [NL * K, D], F32, kind="ExternalOutput").ap()
    muo_r = muo.rearrange("(i k) d -> i k d", k=K)
    lvo_r = lvo.rearrange("(i k) d -> i k d", k=K)

    Exp = mybir.ActivationFunctionType.Exp
    Ln = mybir.ActivationFunctionType.Ln
    Tanh = mybir.ActivationFunctionType.Tanh
    Sqrt = mybir.ActivationFunctionType.Sqrt
    Ident = mybir.ActivationFunctionType.Identity
    Square = mybir.ActivationFunctionType.Square
    AX = mybir.AxisListType.X
    MUL = mybir.AluOpType.mult
    ADD = mybir.AluOpType.add

    with tile.TileContext(nc) as tc:
        with (
            tc.tile_pool(name="const", bufs=1) as cpool,
            tc.tile_pool(name="bigI", bufs=1) as bigI,
            tc.tile_pool(name="bigX", bufs=1) as bigX,
            tc.tile_pool(name="seq", bufs=1) as seq,
        ):
            # ---------- constants ----------
            idn = cpool.tile([128, 128], F32)
            nc.sync.dma_start(idn[:], idnd)
            idnb = cpool.tile([128, 128], BF16)
            nc.sync.dma_start(idnb[:], idnbd)
            onesf = cpool.tile([1, 128], F32R)
            nc.sync.dma_start(onesf[:], onesfd)
            onesb = cpool.tile([128, 1], BF16)
            nc.sync.dma_start(onesb[:], onesbd)
            ind = cpool.tile([K, KW], F32R)
            nc.sync.dma_start(ind[:], indd)
            b1t = cpool.tile([1, H], F32R)
            nc.sync.dma_start(b1t[:], b1r)
            b2t = cpool.tile([1, 256], F32R)
            nc.sync.dma_start(b2t[:], b2r)
            w2t = []
            for j in range(3):
                w2j = cpool.tile([128, 256], F32R, name=f"w2j{j}", tag=f"w2j{j}")
                nc.sync.dma_start(w2j[:], W2p[j])
                w2t.append(w2j)

            # resident items^T (raw, unnormalized; chunked DMAs)
            itemsT = bigI.tile([D, M], F32R)
            for j in range(8):
                sl = slice(j * (M // 8), (j + 1) * (M // 8))
                nc.sync.dma_start(itemsT[:, sl], itemsTd[:, sl])

            # ---------- cores: normalize + column-replicate ----------
            corest = cpool.tile([K, D], F32)
            nc.sync.dma_start(corest[:], coresd)
            css = cpool.tile([K, 1], F32)
            csq = cpool.tile([K, D], F32)
            nc.scalar.activation(csq[:], corest[:], Square, accum_out=css[:])
            cnrm = cpool.tile([K, 1], F32)
            nc.scalar.activation(cnrm[:], css[:], Sqrt)
            cinv = cpool.tile([K, 1], F32)
            nc.vector.reciprocal(cinv[:], cnrm[:])
            coresn = cpool.tile([K, D], F32R)
            nc.vector.tensor_scalar_mul(coresn[:], corest[:], cinv[:])

            ctrep = cpool.tile([D, KW], F32R)  # cores_n[k,:] replicated per block
            with tc.tile_pool(name="prep", bufs=1, space="PSUM") as pp:
                prep = pp.tile([D, KW], F32)
                nc.tensor.matmul(prep[:, 0:512], coresn[:], ind[:, 0:512],
                                 start=True, stop=True)
                nc.tensor.matmul(prep[:, 512:KW], coresn[:],
                                 ind[:, 512:KW], start=True, stop=True)
                nc.scalar.activation(ctrep[:], prep[:], Ident)
            # contiguous [D, K] cores_n^T (strided f32r moving APs are
            # rejected by the ISA checker, so materialize the view)
            coresTn = cpool.tile([D, K + 1], F32R)
            ctrep_k = ctrep.rearrange("p (k i) -> p i k", k=K)[:, 0, :]
            nc.vector.tensor_copy(coresTn[:, 0:K], ctrep_k)
            # pad column (output never read; keep it finite for the sim)
            nc.vector.tensor_copy(coresTn[:, K:K + 1], ctrep_k[:, 0:1])

            # ---------- A1: item row norms (no ACT-table interleave) ----------
            ssq_all = seq.tile([CS, NCH], F32)
            with tc.tile_pool(name="ita", bufs=3) as ita:
                for c in range(NCH):
                    it_c = ita.tile([CS, D], F32, tag="it")
                    nc.sync.dma_start(it_c[:], itemsd[c * CS:(c + 1) * CS, :])
                    sq_c = ita.tile([CS, D], F32, tag="sq")
                    nc.scalar.activation(sq_c[:], it_c[:], Square,
                                         accum_out=ssq_all[:, c:c + 1])
            # sqrt(ss*0.01) = 0.1*||items||; recip -> 10/||items||
            nrm_all = seq.tile([CS, NCH], F32)
            nc.scalar.activation(nrm_all[:], ssq_all[:], Sqrt, scale=0.01)
            invn10 = seq.tile([CS, NCH], F32)
            nc.vector.reciprocal(invn10[:], nrm_all[:])

            # ---------- A2: cates pieces + scaled X^T ----------
            ec_all = seq.tile([CS, NCH * K], F32)   # exp(ct/TAU) unnormalized
            invs_all = seq.tile([CS, NCH], F32)     # 1 / softmax denominator
            xt_dt = BF16 if PH1_BF16 else F32R
            XT = bigX.tile([CS, NCH * NL], xt_dt, tag="big")

            with (
                tc.tile_pool(name="a2", bufs=3) as a2,
                tc.tile_pool(name="pcta", bufs=2, space="PSUM") as pcta,
                tc.tile_pool(name="pxa", bufs=2, space="PSUM") as pxa,
            ):
                for c in range(NCH):
                    ms = slice(c * CS, (c + 1) * CS)
                    pct = pcta.tile([CS, K + 1], F32, tag="pct")
                    nc.tensor.matmul(pct[:], itemsT[:, ms], coresTn[:],
                                     start=True, stop=True)
                    s_c = a2.tile([CS, 1], F32, tag="s")
                    nc.scalar.activation(ec_all[:, c * K:(c + 1) * K],
                                         pct[:, 0:K], Exp,
                                         scale=invn10[:, c:c + 1],
                                         accum_out=s_c[:])
                    nc.vector.reciprocal(invs_all[:, c:c + 1], s_c[:])

                    xt_c = a2.tile([NL, CS], F32, tag="xt")
                    nc.sync.dma_start(xt_c[:], Xl[:, ms])
                    px = pxa.tile([CS, NL], F32, tag="px")
                    nc.tensor.transpose(px[:], xt_c[:], idn[:])
                    # X^T scaled by 1/s[m] while copying out of PSUM
                    nc.scalar.activation(XT[:, c * NL:(c + 1) * NL], px[:],
                                         Ident, scale=invs_all[:, c:c + 1])

            # ---------- phase 1: h_pre[k] = Xk @ W1 + b1 ----------
            h_ks = []
            with (
                tc.tile_pool(name="ph1w", bufs=3) as ph1w,
                tc.tile_pool(name="htile", bufs=1) as htile,
                tc.tile_pool(name="phh", bufs=1, space="PSUM") as phh,
            ):
                ph = []
                for k in range(K):
                    phk = phh.tile([NL, H], F32, name=f"ph{k}", tag=f"ph{k}")
                    nc.tensor.matmul(phk[:], onesf[:], b1t[:],
                                     start=True, stop=False)
                    ph.append(phk)
                for c in range(NCH):
                    w1_c = ph1w.tile([CS, H], w1_dt, tag="w1")
                    nc.sync.dma_start(w1_c[:], W1d[c * CS:(c + 1) * CS, :])
                    for k in range(K):
                        xtk = ph1w.tile([CS, NL], xt_dt, tag="xtk", bufs=3)
                        nc.vector.tensor_scalar_mul(
                            xtk[:], XT[:, c * NL:(c + 1) * NL],
                            ec_all[:, c * K + k:c * K + k + 1])
                        if PH1_BF16:
                            nc.tensor.matmul(ph[k][:], xtk[:], w1_c[:],
                                             start=False, stop=(c == NCH - 1))
                        else:
                            nc.tensor.matmul(ph[k][:], xtk[:], w1_c[:],
                                             start=False, stop=(c == NCH - 1))
                for k in range(K):
                    h_k = htile.tile([NL, H], F32, name=f"h{k}", tag=f"h{k}")
                    nc.scalar.activation(h_k[:], ph[k][:], Tanh)
                    h_ks.append(h_k)

            # ---------- tail: W2, mu/logvar out, z_n ----------
            rhs_dec = seq.tile([D, KW], F32R)
            ssz_all = seq.tile([NL, K], F32)
            muvars = []
            with (
                tc.tile_pool(name="tl", bufs=2) as tl,
                tc.tile_pool(name="tlm", bufs=1) as tlm,
                tc.tile_pool(name="ptt", bufs=2, space="PSUM") as ptt,
                tc.tile_pool(name="pto", bufs=2, space="PSUM") as pto,
            ):
                for k in range(K):
                    h_k = h_ks[k]
                    hT_k = tl.tile([128, 384], F32R, tag="hT")
                    for j in range(3):
                        hj = min(128, H - j * 128)
                        pt = ptt.tile([128, NL], F32, tag="pt")
                        nc.tensor.transpose(
                            pt[0:hj, :], h_k[:, j * 128:j * 128 + hj], idn[:])
                        nc.scalar.activation(
                            hT_k[0:hj, j * 128:(j + 1) * 128], pt[0:hj, :],
                            Ident)
                    po = pto.tile([NL, 256], F32, tag="po")
                    nc.tensor.matmul(po[:], onesf[:], b2t[:],
                                     start=True, stop=False)
                    for j in range(3):
                        hj = min(128, H - j * 128)
                        nc.tensor.matmul(
                            po[:], hT_k[0:hj, j * 128:(j + 1) * 128],
                            w2t[j][0:hj, :], start=False, stop=(j == 2))
                    muvar = tlm.tile([NL, DD], F32, name=f"muvar{k}",
                                     tag=f"muvar{k}")
                    nc.scalar.activation(muvar[:], po[:, 0:DD], Ident)
                    nc.sync.dma_start(muo_r[:, k, :], muvar[:, 0:D])
                    nc.sync.dma_start(lvo_r[:, k, :], muvar[:, D:DD])
                    zsq = tl.tile([NL, D], F32, tag="zsq")
                    nc.scalar.activation(zsq[:], muvar[:, 0:D], Square,
                                         accum_out=ssz_all[:, k:k + 1])
                    muvars.append(muvar)
                nrmz = tl.tile([NL, K], F32, tag="nrmz")
                nc.scalar.activation(nrmz[:], ssz_all[:], Sqrt)
                invz = tl.tile([NL, K], F32, tag="invz")
                nc.vector.reciprocal(invz[:], nrmz[:])
                for k in range(K):
                    z_k = tl.tile([NL, D], F32, tag="z")
                    nc.vector.tensor_scalar_mul(z_k[:], muvars[k][:, 0:D],
                                                invz[:, k:k + 1])
                    pzt = ptt.tile([D, NL], F32, tag="pzt")
                    nc.tensor.transpose(pzt[:], z_k[:], idn[:])
                    nc.scalar.activation(rhs_dec[:, k * NL:(k + 1) * NL],
                                         pzt[:], Ident)
                nc.vector.tensor_add(rhs_dec[:], rhs_dec[:], ctrep[:])

            lninvs = seq.tile([CS, NCH], F32)
            nc.scalar.activation(lninvs[:], invs_all[:], Ln)

            # ---------- decoder sweep ----------
            probsT = bigX.tile([CS, NCH * NL], BF16, tag="big")
            invS = seq.tile([NL, 1], F32)
            with (
                tc.tile_pool(name="dec", bufs=3) as dec,
                tc.tile_pool(name="pld", bufs=2, space="PSUM") as pld,
                tc.tile_pool(name="psd", bufs=1, space="PSUM") as psd,
            ):
                pS = psd.tile([1, NL], F32, tag="pS")
                for c in range(NCH):
                    ms = slice(c * CS, (c + 1) * CS)
                    pL = pld.tile([CS, KW], F32, tag="pL")
                    nc.tensor.matmul(pL[:, 0:512], itemsT[:, ms],
                                     rhs_dec[:, 0:512], start=True,
                                     stop=True)
                    nc.tensor.matmul(pL[:, 512:KW], itemsT[:, ms],
                                     rhs_dec[:, 512:KW], start=True,
                                     stop=True)
                    E_c = dec.tile([CS, KW], BF16, tag="E")
                    nc.scalar.activation(E_c[:], pL[:], Exp,
                                         scale=invn10[:, c:c + 1],
                                         bias=lninvs[:, c:c + 1])
                    Ek = E_c.rearrange("p (k i) -> p i k", k=K)
                    with nc.allow_low_precision("probs k-sum to bf16"):
                        nc.vector.tensor_reduce(
                            probsT[:, c * NL:(c + 1) * NL], Ek, axis=AX,
                            op=ADD)
                    nc.tensor.matmul(pS[:], onesb[0:CS, :],
                                     probsT[:, c * NL:(c + 1) * NL],
                                     start=(c == 0), stop=(c == NCH - 1))

                # S -> 1/S per batch row
                s_row = dec.tile([1, NL], F32, tag="srow")
                nc.scalar.activation(s_row[:], pS[:], Ident)
                pSt = pld.tile([NL, 1], F32, tag="pSt", bufs=1)
                nc.tensor.transpose(pSt[:], s_row[:], idn[0:1, 0:1])
                nc.vector.reciprocal(invS[:], pSt[:])

            # ---------- final: logits = ln(probs * invS) ----------
            with (
                tc.tile_pool(name="fin", bufs=3) as fin,
                tc.tile_pool(name="pfin", bufs=2, space="PSUM") as pfin,
            ):
                for c in range(NCH):
                    pf = pfin.tile([NL, CS], BF16, tag="pf")
                    nc.tensor.transpose(pf[:], probsT[:, c * NL:(c + 1) * NL],
                                        idnb[0:CS, 0:CS])
                    lg_c = fin.tile([NL, CS], F32, tag="lg")
                    nc.scalar.activation(lg_c[:], pf[:], Ln, scale=invS[:])
                    nc.sync.dma_start(lg[:, c * CS:(c + 1) * CS], lg_c[:])

    nc.compile()
    return nc


_CACHE = {}


def _get_nc():
    if "nc" not in _CACHE:
        _CACHE["nc"] = build_nc()
    return _CACHE["nc"]


def make_in_maps(X, A, W1, b1, W2, b2, items, cores):
    X = np.ascontiguousarray(X, dtype=np.float32)
    W1 = np.ascontiguousarray(W1, dtype=np.float32)
    items = np.ascontiguousarray(items, dtype=np.float32)
    cores = np.ascontiguousarray(cores, dtype=np.float32)

    w1_host = W1.astype(ml_dtypes.bfloat16) if PH1_BF16 else W1
    W2p = np.zeros((3, 128, 256), dtype=np.float32)
    for j in range(3):
        hj = min(128, H - j * 128)
        W2p[j, :hj, :DD] = W2[j * 128:j * 128 + hj, :]
    b2p = np.zeros((1, 256), dtype=np.float32)
    b2p[0, :DD] = b2
    ind = np.zeros((K, KW), dtype=np.float32)
    for k in range(K):
        ind[k, k * NL:(k + 1) * NL] = 1.0

    common = {
        "W1d": w1_host, "b1r": b1.reshape(1, H).astype(np.float32),
        "W2p": W2p, "b2r": b2p,
        "itemsd": items, "itemsTd": np.ascontiguousarray(items.T),
        "coresd": cores,
        "idnd": np.eye(128, dtype=np.float32),
        "idnbd": np.eye(128).astype(ml_dtypes.bfloat16),
        "onesfd": np.ones((1, 128), dtype=np.float32),
        "onesbd": np.ones((128, 1)).astype(ml_dtypes.bfloat16),
        "indd": ind,
    }
    return [dict(common, Xl=X[c * NL:(c + 1) * NL]) for c in range(N_CORES)]


def run(in_maps, **kwargs):
    nc = _get_nc()
    res = run_bass_kernel_spmd(nc, in_maps, core_ids=list(range(N_CORES)),
                               **kwargs)
    logits = np.concatenate([res.results[c]["lg"] for c in range(N_CORES)], 0)
    mu = np.concatenate([res.results[c]["muo"] for c in range(N_CORES)], 0)
    logvar = np.concatenate(
        [res.results[c]["lvo"] for c in range(N_CORES)], 0)
    return (logits, mu, logvar), res


def kernel(X, A, W1, b1, W2, b2, items, cores):
    out, _ = run(make_in_maps(X, A, W1, b1, W2, b2, items, cores))
    return out
